# revision 12
# baseline (speedup 1.0000x reference)
"""BigBird attention Trainium2 kernel (Bass/Tile), 8-core SPMD.

Sharding: core c -> (batch b = c//4, sequence quarter t = c%4).
Each core computes ALL 16 heads for its 2048 "own" local tokens, plus a
1-block (128 token) halo on each side (recomputed locally, circular) and
the 16 global tokens.  Outputs are disjoint rows of y, so the host gather
is pure concatenation.  The only cross-core communication is a 66 KB
AllReduce of the global-query attention partial sums (numerator+denominator).

Device x column layout per core (2320 cols): [own 2048 | hl 128 | hr 128 | g 16].
"""

import os
import numpy as np

# ---------------- problem constants (hardcoded per contract) ----------------
D_MODEL = 1024
H = 16
DK = 64
DV = 64
BLOCK = 128
G = 16
B = 2
T = G + 8192          # 8208
NBLK = 64             # local blocks per batch
QB = 16               # own q blocks per core
T_OWN = QB * BLOCK    # 2048
XC = T_OWN + 2 * BLOCK + G  # 2320 device x cols: [own | hl | hr | g]
N_CORES = 8
P = 128
KC = D_MODEL // P     # 8 contraction chunks
MC = (H * DK) // P    # 8 row chunks of qT/kT (2 heads per chunk)
SCALE = 1.0 / 8.0     # 1/sqrt(64)

# dtype knobs
USE_F32R = os.environ.get("BB_NO_F32R", "") == ""     # fp32r matmuls for fp32 data
ATT_BF16 = os.environ.get("BB_ATT_F32", "") == ""     # bf16 q/k/v/probs/out_x storage

# column offsets in the device-x layout
OWN0 = 0
HL0 = T_OWN            # 2048
HR0 = T_OWN + BLOCK    # 2176
G0 = T_OWN + 2 * BLOCK # 2304 (globals in kT / x layout)
QXC = T_OWN + G        # 2064 qT cols: [own | g]
QG0 = T_OWN            # globals offset within qT


def _kcols(r):
    """Columns of k-block with relative index r in [-1, 16]."""
    if r == -1:
        return HL0
    if r == 16:
        return HR0
    return r * BLOCK


def _vblk(r):
    """v_sb block index for relative k-block r."""
    if r == -1:
        return 16
    if r == 16:
        return 17
    return r


def build_program():
    import concourse.bacc as bacc
    import concourse.tile as tile
    import concourse.mybir as mybir
    from concourse.masks import make_identity
    from contextlib import ExitStack

    dt = mybir.dt
    F32 = dt.float32
    ATT = dt.bfloat16 if ATT_BF16 else dt.float32
    MMDT = dt.float32r if USE_F32R else dt.float32
    R32 = MMDT
    Exp = mybir.ActivationFunctionType.Exp
    Copy = mybir.ActivationFunctionType.Copy

    nc = bacc.Bacc("TRN2", target_bir_lowering=False, debug=False,
                   num_devices=N_CORES)

    def rb(ap):  # bitcast an fp32 AP (e.g. DRAM input view) to float32r
        return ap.bitcast(R32) if USE_F32R else ap

    # ---------------- external I/O ----------------
    xT_d = nc.dram_tensor("xin", [P, KC, XC], F32, kind="ExternalInput").ap()
    wqT_d = nc.dram_tensor("wqT", [P, KC, H * DK], F32, kind="ExternalInput").ap()
    wkT_d = nc.dram_tensor("wkT", [P, KC, H * DK], F32, kind="ExternalInput").ap()
    wvT_d = nc.dram_tensor("wvT", [P, KC, H * DV], F32, kind="ExternalInput").ap()
    woT_d = nc.dram_tensor("woT", [P, KC, D_MODEL], F32, kind="ExternalInput").ap()
    bo_d = nc.dram_tensor("bo", [1, D_MODEL], F32, kind="ExternalInput").ap()
    y_own_d = nc.dram_tensor("y_own", [T_OWN, D_MODEL], F32,
                             kind="ExternalOutput").ap()
    y_g_d = nc.dram_tensor("y_g", [G, D_MODEL], F32, kind="ExternalOutput").ap()

    with tile.TileContext(nc) as tc, ExitStack() as top:
        # ------------- persistent SBUF -------------
        pool_qT = top.enter_context(tc.tile_pool(name="qT", bufs=1))
        pool_kT = top.enter_context(tc.tile_pool(name="kT", bufs=1))
        pool_v = top.enter_context(tc.tile_pool(name="v", bufs=1))
        pool_misc = top.enter_context(tc.tile_pool(name="misc", bufs=1))
        pool_outx = top.enter_context(tc.tile_pool(name="outx", bufs=1))
        out_x = pool_outx.tile([P, QB, H * DV], ATT)
        qT_sb = pool_qT.tile([P, MC, QXC], ATT)       # rows (h,d) chunked, cols t
        kT_sb = pool_kT.tile([P, MC, XC], ATT)
        v_sb = pool_v.tile([P, 18, H, 66], ATT)      # [row%128, kblk, h, d(+1)]
        vg_sb = pool_misc.tile([G, H, 66], ATT)      # global v rows
        gx_sb = pool_misc.tile([DV + 1, H, G], F32)  # gx partials [d(+den), h, g]
        nc.gpsimd.memset(v_sb[:, :, :, 64:65], 1.0)
        nc.gpsimd.memset(vg_sb[:, :, 64:65], 1.0)

        # DRAM bounce buffers for the gx AllReduce
        pool_dram = top.enter_context(tc.tile_pool(name="dram", bufs=1, space="DRAM"))
        gx_part_d = pool_dram.tile([DV + 1, H, G], F32)
        gx_full_d = pool_dram.tile([DV + 1, H, G], F32)

        # ---------------- phase 1a: q,k projections ----------------
        NW = 384

        def _segments(pairs):
            """Split (src0, dst0, width) pairs into <=NW chunks."""
            out = []
            for src0, dst0, width in pairs:
                o = 0
                while o < width:
                    w = min(NW, width - o)
                    out.append((src0 + o, dst0 + o, w))
                    o += w
            return out

        q_segs = _segments([(OWN0, 0, T_OWN), (G0, QG0, G)])
        k_segs = _segments([(0, 0, XC)])
        for pname, w_d, dst, segs, use_act in (("q", wqT_d, qT_sb, q_segs, True),
                                               ("k", wkT_d, kT_sb, k_segs, False)):
            with ExitStack() as s1:
                pool_w1 = s1.enter_context(tc.tile_pool(name=f"w1{pname}", bufs=1))
                pool_x1 = s1.enter_context(tc.tile_pool(name=f"x1{pname}", bufs=2))
                pool_ps1 = s1.enter_context(
                    tc.tile_pool(name=f"ps1{pname}", bufs=4, space="PSUM"))
                w_sb = pool_w1.tile([P, KC, H * DK], R32, name=f"w_{pname}")
                nc.sync.dma_start(out=w_sb[:], in_=rb(w_d[:]))
                for src0, dst0, nw in segs:
                    xt = pool_x1.tile([P, KC, NW], R32, tag="xt", name="xt")
                    nc.sync.dma_start(out=xt[:, :, :nw],
                                      in_=rb(xT_d[:, :, src0:src0 + nw]))
                    for mc in range(MC):
                        ps = pool_ps1.tile([P, NW], F32, tag="ps1", name="ps")
                        for kc in range(KC):
                            nc.tensor.matmul(
                                ps[:, :nw],
                                lhsT=w_sb[:, kc, mc * P:(mc + 1) * P],
                                rhs=xt[:, kc, :nw],
                                start=(kc == 0), stop=(kc == KC - 1))
                        if use_act:
                            nc.scalar.activation(dst[:, mc, dst0:dst0 + nw],
                                                 ps[:, :nw], Copy)
                        else:
                            nc.vector.tensor_copy(dst[:, mc, dst0:dst0 + nw],
                                                  ps[:, :nw])

        # ---------------- phase 1b: v projection ----------------
        with ExitStack() as s2:
            pool_w2 = s2.enter_context(tc.tile_pool(name="w2", bufs=1))
            pool_x2 = s2.enter_context(tc.tile_pool(name="x2", bufs=3))
            pool_ps2 = s2.enter_context(tc.tile_pool(name="ps2", bufs=3, space="PSUM"))
            wv_sb = pool_w2.tile([P, KC, H * DV], R32)
            nc.sync.dma_start(out=wv_sb[:], in_=rb(wvT_d[:]))
            for m in range(19):           # 18 local blocks + globals(16 rows)
                rows = P if m < 18 else G
                xt2 = pool_x2.tile([P, KC, P], R32, tag="xt2")
                nc.sync.dma_start(out=xt2[:, :, :rows],
                                  in_=rb(xT_d[:, :, m * P:m * P + rows]))
                for nv in range(2):       # v inner-dim halves (8 heads each)
                    ps = pool_ps2.tile([P, 512], F32, tag="ps2")
                    for kc in range(KC):
                        nc.tensor.matmul(
                            ps[:rows, :],
                            lhsT=xt2[:, kc, :rows],
                            rhs=wv_sb[:, kc, nv * 512:(nv + 1) * 512],
                            start=(kc == 0), stop=(kc == KC - 1))
                    src = ps[:rows, :].rearrange("p (h d) -> p h d", h=8)
                    if m < 18:
                        dst = v_sb[:rows, m, nv * 8:(nv + 1) * 8, 0:64]
                    else:
                        dst = vg_sb[:rows, nv * 8:(nv + 1) * 8, 0:64]
                    nc.vector.tensor_copy(dst, src)

        # ---------------- phase 2: attention ----------------
        with ExitStack() as s3:
            pool_probs = s3.enter_context(tc.tile_pool(name="probs", bufs=5))
            pool_pxg = s3.enter_context(tc.tile_pool(name="pxg", bufs=2))
            pool_ps_s = s3.enter_context(tc.tile_pool(name="ps_s", bufs=2, space="PSUM"))
            pool_ps_o = s3.enter_context(tc.tile_pool(name="ps_o", bufs=2, space="PSUM"))
            pool_ps_gx = s3.enter_context(tc.tile_pool(name="ps_gx", bufs=1, space="PSUM"))
            pool_ps_xg = s3.enter_context(tc.tile_pool(name="ps_xg", bufs=1, space="PSUM"))
            pool_nrm = s3.enter_context(tc.tile_pool(name="nrm", bufs=3))

            for h in range(H):
                hp, hb = h // 2, 64 * (h % 2)
                qk = lambda sb, c0, c1: sb[hb:hb + 64, hp, c0:c1]

                # xg scores (local q vs global k), k-major [16, 2048]
                pxg = pool_pxg.tile([G, T_OWN], ATT, tag="pxg")
                for nq in range(4):
                    psx = pool_ps_xg.tile([G, 512], F32, tag="psxg")
                    nc.tensor.matmul(psx[:, :],
                                     lhsT=qk(kT_sb, G0, G0 + G),
                                     rhs=qk(qT_sb, nq * 512, (nq + 1) * 512),
                                     start=True, stop=True)
                    nc.scalar.activation(pxg[:, nq * 512:(nq + 1) * 512],
                                         psx[:, :], Exp, scale=SCALE)

                ps_gx = pool_ps_gx.tile([DV + 1, G], F32, tag="psgx")
                probs = {}
                for r_ in range(-1, 17):
                    # local scores for q blocks in window of k-block r_
                    ilo, ihi = max(r_ - 1, 0), min(r_ + 1, QB - 1)
                    nloc = (ihi - ilo + 1) * BLOCK
                    own = 0 <= r_ <= 15
                    ntot = nloc + (G if own else 0)
                    kc0 = _kcols(r_)
                    ps_s = pool_ps_s.tile([P, 400], F32, tag="ps_s")
                    nc.tensor.matmul(ps_s[:, :nloc],
                                     lhsT=qk(kT_sb, kc0, kc0 + BLOCK),
                                     rhs=qk(qT_sb, ilo * BLOCK, (ihi + 1) * BLOCK),
                                     start=True, stop=True)
                    if own:  # gx scores appended (global q vs this k-block)
                        nc.tensor.matmul(ps_s[:, nloc:ntot],
                                         lhsT=qk(kT_sb, kc0, kc0 + BLOCK),
                                         rhs=qk(qT_sb, QG0, QG0 + G),
                                         start=True, stop=True)
                    pt = pool_probs.tile([P, 400], ATT, tag="probs")
                    nc.scalar.activation(pt[:, :ntot], ps_s[:, :ntot],
                                         Exp, scale=SCALE)
                    probs[r_] = (pt, ilo)
                    if own:  # accumulate gx numerator/denominator over own blocks
                        nc.tensor.matmul(ps_gx[:, :],
                                         lhsT=v_sb[:, r_, h, 0:65],
                                         rhs=pt[:, nloc:ntot],
                                         start=(r_ == 0), stop=(r_ == 15))

                    i = r_ - 1  # q-block whose window is now complete
                    if 0 <= i <= QB - 1:
                        ps_o = pool_ps_o.tile([P, 130], F32, tag="ps_o")
                        for dj, j in enumerate((i - 1, i, i + 1)):
                            pj, jlo = probs[j]
                            c0 = (i - jlo) * BLOCK
                            nc.tensor.matmul(ps_o[:, 0:65],
                                             lhsT=pj[:, c0:c0 + BLOCK],
                                             rhs=v_sb[:, _vblk(j), h, 0:65],
                                             start=(dj == 0), stop=(dj == 2))
                        nc.tensor.matmul(ps_o[:, 65:130],
                                         lhsT=pxg[:, i * BLOCK:(i + 1) * BLOCK],
                                         rhs=vg_sb[:, h, 0:65],
                                         start=True, stop=True)
                        rec = pool_nrm.tile([P, 2], F32, tag="rec")
                        nc.vector.reciprocal(rec[:, 0:1], ps_o[:, 64:65])
                        nc.vector.reciprocal(rec[:, 1:2], ps_o[:, 129:130])
                        tL = pool_nrm.tile([P, DV], ATT, tag="tL")
                        tG = pool_nrm.tile([P, DV], ATT, tag="tG")
                        nc.scalar.activation(tL[:], ps_o[:, 0:64], Copy,
                                             scale=rec[:, 0:1])
                        nc.scalar.activation(tG[:], ps_o[:, 65:129], Copy,
                                             scale=rec[:, 1:2])
                        nc.vector.tensor_add(
                            out_x[:, i, h * DV:(h + 1) * DV], tL[:], tG[:])
                        probs.pop(i - 1, None)
                # stash gx partials for this head
                nc.vector.tensor_copy(gx_sb[:, h, :], ps_gx[:, :])

            nc.sync.dma_start(out=gx_part_d[:], in_=gx_sb[:])
            nc.gpsimd.collective_compute(
                "AllReduce", mybir.AluOpType.add,
                replica_groups=[[0, 1, 2, 3], [4, 5, 6, 7]],
                ins=[gx_part_d.opt()], outs=[gx_full_d.opt()])

        # ---------------- phase 3: output projection ----------------
        with ExitStack() as s4:
            pool_wo = s4.enter_context(tc.tile_pool(name="wo", bufs=1))
            pool_ot = s4.enter_context(tc.tile_pool(name="ot", bufs=10))
            pool_pst = s4.enter_context(tc.tile_pool(name="pst", bufs=2, space="PSUM"))
            pool_psy = s4.enter_context(tc.tile_pool(name="psy", bufs=2, space="PSUM"))
            pool_ysb = s4.enter_context(tc.tile_pool(name="ysb", bufs=3))
            pool_gxf = s4.enter_context(tc.tile_pool(name="gxf", bufs=1))
            wo_sb = pool_wo.tile([P, KC, D_MODEL], R32)
            bo_sb = pool_wo.tile([1, D_MODEL], F32)
            ones1 = pool_wo.tile([1, P], F32)
            bias_sb = pool_wo.tile([P, D_MODEL], F32)
            ident = pool_wo.tile([P, P], ATT)
            nc.sync.dma_start(out=wo_sb[:], in_=rb(woT_d[:]))
            nc.sync.dma_start(out=bo_sb[:], in_=bo_d[:])
            nc.vector.memset(ones1[:], 1.0)
            make_identity(nc, ident[:])
            for nv in range(2):
                psb0 = pool_psy.tile([P, 512], F32, tag="psy")
                nc.tensor.matmul(psb0[:], lhsT=ones1[:],
                                 rhs=bo_sb[:, nv * 512:(nv + 1) * 512],
                                 start=True, stop=True)
                nc.scalar.activation(bias_sb[:, nv * 512:(nv + 1) * 512],
                                     psb0[:], Copy)

            for m in range(QB):
                ots = []
                for kc in range(KC):
                    pst = pool_pst.tile([P, P], ATT, tag="pst")
                    nc.tensor.transpose(pst[:],
                                        out_x[:, m, kc * P:(kc + 1) * P],
                                        ident[:])
                    ot = pool_ot.tile([P, P], R32, tag="ot")
                    nc.vector.tensor_copy(ot[:], pst[:])
                    ots.append(ot)
                for nv in range(2):
                    psy = pool_psy.tile([P, 512], F32, tag="psy")
                    for kc in range(KC):
                        nc.tensor.matmul(psy[:],
                                         lhsT=ots[kc][:],
                                         rhs=wo_sb[:, kc, nv * 512:(nv + 1) * 512],
                                         start=(kc == 0), stop=(kc == KC - 1))
                    ysb = pool_ysb.tile([P, 512], F32, tag="ysb")
                    nc.vector.tensor_add(ysb[:], psy[:],
                                         bias_sb[:, nv * 512:(nv + 1) * 512])
                    nc.sync.dma_start(
                        out=y_own_d[m * P:(m + 1) * P, nv * 512:(nv + 1) * 512],
                        in_=ysb[:])

            # ----- global rows: normalize gx and project -----
            num_sb = pool_gxf.tile([P, KC, G], F32)     # [(h d) chunks, g]
            den_sb = pool_gxf.tile([H, G], F32)
            rden = pool_gxf.tile([H, G], F32)
            sel = pool_gxf.tile([H, H * 64], F32)
            norm_sb = pool_gxf.tile([P, KC, G], R32)
            nc.gpsimd.memset(sel[:], 0.0)
            sel3 = sel[:].rearrange("k (h d) -> k h d", h=H)
            nc.gpsimd.affine_select(
                out=sel3, in_=sel3,
                compare_op=mybir.AluOpType.not_equal, fill=1.0,
                base=0, pattern=[[-1, H], [0, 64]], channel_multiplier=1)
            for h in range(H):
                nc.sync.dma_start(
                    out=num_sb[64 * (h % 2):64 * (h % 2) + 64, h // 2, :],
                    in_=gx_full_d[0:64, h, :])
            nc.sync.dma_start(out=den_sb[:], in_=gx_full_d[64, :, :])
            nc.vector.reciprocal(rden[:], den_sb[:])
            for h in range(H):
                psb = pool_pst.tile([64, G], F32, tag="pst")
                nc.tensor.matmul(psb[:], lhsT=sel[:, h * 64:(h + 1) * 64],
                                 rhs=rden[:], start=True, stop=True)
                sl = (slice(64 * (h % 2), 64 * (h % 2) + 64), h // 2, slice(None))
                nc.vector.tensor_mul(norm_sb[sl], num_sb[sl], psb[:])
            for nv in range(2):
                psy = pool_psy.tile([G, 512], F32, tag="psy")
                for kc in range(KC):
                    nc.tensor.matmul(psy[:],
                                     lhsT=norm_sb[:, kc, :],
                                     rhs=wo_sb[:, kc, nv * 512:(nv + 1) * 512],
                                     start=(kc == 0), stop=(kc == KC - 1))
                ygsb = pool_ysb.tile([G, 512], F32, tag="ygsb")
                nc.vector.tensor_add(ygsb[:], psy[:],
                                     bias_sb[0:G, nv * 512:(nv + 1) * 512])
                nc.sync.dma_start(out=y_g_d[:, nv * 512:(nv + 1) * 512],
                                  in_=ygsb[:])

    nc.compile()
    return nc


def shard_inputs(x, Wq, Wk, Wv, Wo, bo):
    """Build the 8 per-core input maps."""
    x = np.asarray(x, dtype=np.float32)
    wqT = np.ascontiguousarray(
        np.asarray(Wq, np.float32).T.reshape(KC, P, H * DK).transpose(1, 0, 2))
    wkT = np.ascontiguousarray(
        np.asarray(Wk, np.float32).T.reshape(KC, P, H * DK).transpose(1, 0, 2))
    wvT = np.ascontiguousarray(
        np.asarray(Wv, np.float32).T.reshape(KC, P, H * DV).transpose(1, 0, 2))
    woT = np.ascontiguousarray(
        np.asarray(Wo, np.float32).T.reshape(KC, P, D_MODEL).transpose(1, 0, 2))
    bo2 = np.asarray(bo, np.float32).reshape(1, D_MODEL)
    in_maps = []
    for c in range(N_CORES):
        b, t = c // 4, c % 4
        xg = x[b, :G]                       # [16, 1024]
        xl = x[b, G:]                       # [8192, 1024]
        own = xl[t * T_OWN:(t + 1) * T_OWN]
        hl = xl[((16 * t - 1) % NBLK) * BLOCK:][:BLOCK]
        hr = xl[((16 * t + 16) % NBLK) * BLOCK:][:BLOCK]
        xc = np.concatenate([own, hl, hr, xg], axis=0)          # [2320, 1024]
        xT = np.ascontiguousarray(
            xc.T.reshape(KC, P, XC).transpose(1, 0, 2))         # [128, 8, 2320]
        in_maps.append({"xin": xT, "wqT": wqT, "wkT": wkT, "wvT": wvT,
                        "woT": woT, "bo": bo2})
    return in_maps


_NC_CACHE = {}


def get_program():
    key = (USE_F32R, ATT_BF16)
    if key not in _NC_CACHE:
        _NC_CACHE[key] = build_program()
    return _NC_CACHE[key]


def _install_ntff_hook():
    """Provide antenv.axon_hooks (missing in this image) so that
    run_bass_kernel_spmd(trace=True) can capture NTFF profiles."""
    import sys, types
    if "antenv.axon_hooks" in sys.modules:
        return
    try:
        import antenv  # noqa: F401
        from trn_agent_boot.trn_boot import _ntff_profile_via_ctypes
        mod = types.ModuleType("antenv.axon_hooks")
        mod._hook = _ntff_profile_via_ctypes("/opt/axon/libaxon_pjrt.so")
        mod.set_axon_ntff_profile_hook = lambda h: setattr(mod, "_hook", h)
        mod.get_axon_ntff_profile_hook = lambda: mod._hook
        sys.modules["antenv.axon_hooks"] = mod
    except Exception as e:  # profiling is optional
        print(f"ntff hook install failed: {e}")


def run(x, Wq, Wk, Wv, Wo, bo, trace=False):
    from concourse.bass_utils import run_bass_kernel_spmd
    if trace:
        _install_ntff_hook()
    nc = get_program()
    in_maps = shard_inputs(x, Wq, Wk, Wv, Wo, bo)
    res = run_bass_kernel_spmd(nc, in_maps, list(range(N_CORES)), trace=trace)
    y = np.empty((B, T, D_MODEL), dtype=np.float32)
    for c in range(N_CORES):
        b, t = c // 4, c % 4
        if t == 0:
            y[b, :G] = res.results[c]["y_g"]
        y[b, G + t * T_OWN:G + (t + 1) * T_OWN] = res.results[c]["y_own"]
    return y, res


def kernel(x, Wq, Wk, Wv, Wo, bo):
    y, _ = run(x, Wq, Wk, Wv, Wo, bo, trace=False)
    return y


# revision 13
# speedup vs baseline: 1.1002x; 1.1002x over previous
"""BigBird attention Trainium2 kernel (Bass/Tile), 8-core SPMD.

Sharding: core c -> (batch b = c//4, sequence quarter t = c%4).
Each core computes ALL 16 heads for its 2048 "own" local tokens, plus a
1-block (128 token) halo on each side (recomputed locally, circular) and
the 16 global tokens.  Outputs are disjoint rows of y, so the host gather
is pure concatenation.  The only cross-core communication is a 66 KB
AllReduce of the global-query attention partial sums (numerator+denominator).

Device x column layout per core (2320 cols): [own 2048 | hl 128 | hr 128 | g 16].
"""

import os
import numpy as np

# ---------------- problem constants (hardcoded per contract) ----------------
D_MODEL = 1024
H = 16
DK = 64
DV = 64
BLOCK = 128
G = 16
B = 2
T = G + 8192          # 8208
NBLK = 64             # local blocks per batch
QB = 16               # own q blocks per core
T_OWN = QB * BLOCK    # 2048
XC = T_OWN + 2 * BLOCK + G  # 2320 device x cols: [own | hl | hr | g]
N_CORES = 8
P = 128
KC = D_MODEL // P     # 8 contraction chunks
MC = (H * DK) // P    # 8 row chunks of qT/kT (2 heads per chunk)
SCALE = 1.0 / 8.0     # 1/sqrt(64)

# dtype knobs
USE_F32R = os.environ.get("BB_NO_F32R", "") == ""     # fp32r matmuls for fp32 data
ATT_BF16 = os.environ.get("BB_ATT_F32", "") == ""     # bf16 q/k/v/probs/out_x storage

# column offsets in the device-x layout
OWN0 = 0
HL0 = T_OWN            # 2048
HR0 = T_OWN + BLOCK    # 2176
G0 = T_OWN + 2 * BLOCK # 2304 (globals in kT / x layout)
QXC = T_OWN + G        # 2064 qT cols: [own | g]
QG0 = T_OWN            # globals offset within qT


def _kcols(r):
    """Columns of k-block with relative index r in [-1, 16]."""
    if r == -1:
        return HL0
    if r == 16:
        return HR0
    return r * BLOCK


def _vblk(r):
    """v_sb block index for relative k-block r."""
    if r == -1:
        return 16
    if r == 16:
        return 17
    return r


def build_program():
    import concourse.bacc as bacc
    import concourse.tile as tile
    import concourse.mybir as mybir
    from concourse.masks import make_identity
    from contextlib import ExitStack

    dt = mybir.dt
    F32 = dt.float32
    ATT = dt.bfloat16 if ATT_BF16 else dt.float32
    MMDT = dt.float32r if USE_F32R else dt.float32
    R32 = MMDT
    Exp = mybir.ActivationFunctionType.Exp
    Copy = mybir.ActivationFunctionType.Copy

    nc = bacc.Bacc("TRN2", target_bir_lowering=False, debug=False,
                   num_devices=N_CORES)

    def rb(ap):  # bitcast an fp32 AP (e.g. DRAM input view) to float32r
        return ap.bitcast(R32) if USE_F32R else ap

    # ---------------- external I/O ----------------
    xT_d = nc.dram_tensor("xin", [P, KC, XC], F32, kind="ExternalInput").ap()
    wqT_d = nc.dram_tensor("wqT", [P, KC, H * DK], F32, kind="ExternalInput").ap()
    wkT_d = nc.dram_tensor("wkT", [P, KC, H * DK], F32, kind="ExternalInput").ap()
    wvT_d = nc.dram_tensor("wvT", [P, KC, H * DV], F32, kind="ExternalInput").ap()
    woT_d = nc.dram_tensor("woT", [P, KC, D_MODEL], F32, kind="ExternalInput").ap()
    bo_d = nc.dram_tensor("bo", [1, D_MODEL], F32, kind="ExternalInput").ap()
    y_own_d = nc.dram_tensor("y_own", [T_OWN, D_MODEL], F32,
                             kind="ExternalOutput").ap()
    y_g_d = nc.dram_tensor("y_g", [G, D_MODEL], F32, kind="ExternalOutput").ap()

    with tile.TileContext(nc) as tc, ExitStack() as top:
        # ------------- persistent SBUF -------------
        pool_qT = top.enter_context(tc.tile_pool(name="qT", bufs=1))
        pool_kT = top.enter_context(tc.tile_pool(name="kT", bufs=1))
        pool_v = top.enter_context(tc.tile_pool(name="v", bufs=1))
        pool_misc = top.enter_context(tc.tile_pool(name="misc", bufs=1))
        pool_outx = top.enter_context(tc.tile_pool(name="outx", bufs=1))
        out_x = pool_outx.tile([P, QB, H * DV], ATT)
        qT_sb = pool_qT.tile([P, MC, QXC], ATT)       # rows (h,d) chunked, cols t
        kT_sb = pool_kT.tile([P, MC, XC], ATT)
        v_sb = pool_v.tile([P, 18, H, 66], ATT)      # [row%128, kblk, h, d(+1)]
        vg_sb = pool_misc.tile([G, H, 66], ATT)      # global v rows
        gx_sb = pool_misc.tile([DV + 1, H, G], F32)  # gx partials [d(+den), h, g]
        nc.gpsimd.memset(v_sb[:, :, :, 64:65], 1.0)
        nc.gpsimd.memset(vg_sb[:, :, 64:65], 1.0)

        # DRAM bounce buffers for the gx AllReduce
        pool_dram = top.enter_context(tc.tile_pool(name="dram", bufs=1, space="DRAM"))
        gx_part_d = pool_dram.tile([DV + 1, H, G], F32)
        gx_full_d = pool_dram.tile([DV + 1, H, G], F32)

        # ---------------- phase 1a: q,k projections ----------------
        NW = 384

        def _segments(pairs):
            """Split (src0, dst0, width) pairs into <=NW chunks."""
            out = []
            for src0, dst0, width in pairs:
                o = 0
                while o < width:
                    w = min(NW, width - o)
                    out.append((src0 + o, dst0 + o, w))
                    o += w
            return out

        q_segs = _segments([(OWN0, 0, T_OWN), (G0, QG0, G)])
        k_segs = _segments([(0, 0, XC)])
        for pname, w_d, dst, segs, use_act in (("q", wqT_d, qT_sb, q_segs, True),
                                               ("k", wkT_d, kT_sb, k_segs, False)):
            with ExitStack() as s1:
                pool_w1 = s1.enter_context(tc.tile_pool(name=f"w1{pname}", bufs=1))
                pool_x1 = s1.enter_context(tc.tile_pool(name=f"x1{pname}", bufs=2))
                pool_ps1 = s1.enter_context(
                    tc.tile_pool(name=f"ps1{pname}", bufs=4, space="PSUM"))
                w_sb = pool_w1.tile([P, KC, H * DK], R32, name=f"w_{pname}")
                nc.sync.dma_start(out=w_sb[:], in_=rb(w_d[:]))
                for src0, dst0, nw in segs:
                    xt = pool_x1.tile([P, KC, NW], R32, tag="xt", name="xt")
                    nc.sync.dma_start(out=xt[:, :, :nw],
                                      in_=rb(xT_d[:, :, src0:src0 + nw]))
                    for mc in range(MC):
                        ps = pool_ps1.tile([P, NW], F32, tag="ps1", name="ps")
                        for kc in range(KC):
                            nc.tensor.matmul(
                                ps[:, :nw],
                                lhsT=w_sb[:, kc, mc * P:(mc + 1) * P],
                                rhs=xt[:, kc, :nw],
                                start=(kc == 0), stop=(kc == KC - 1))
                        if use_act:
                            nc.scalar.activation(dst[:, mc, dst0:dst0 + nw],
                                                 ps[:, :nw], Copy)
                        else:
                            nc.vector.tensor_copy(dst[:, mc, dst0:dst0 + nw],
                                                  ps[:, :nw])

        # ---------------- phase 1b: v projection ----------------
        with ExitStack() as s2:
            pool_w2 = s2.enter_context(tc.tile_pool(name="w2", bufs=1))
            pool_x2 = s2.enter_context(tc.tile_pool(name="x2", bufs=3))
            pool_ps2 = s2.enter_context(tc.tile_pool(name="ps2", bufs=3, space="PSUM"))
            wv_sb = pool_w2.tile([P, KC, H * DV], R32)
            nc.sync.dma_start(out=wv_sb[:], in_=rb(wvT_d[:]))
            for m in range(19):           # 18 local blocks + globals(16 rows)
                rows = P if m < 18 else G
                xt2 = pool_x2.tile([P, KC, P], R32, tag="xt2")
                nc.sync.dma_start(out=xt2[:, :, :rows],
                                  in_=rb(xT_d[:, :, m * P:m * P + rows]))
                for nv in range(2):       # v inner-dim halves (8 heads each)
                    ps = pool_ps2.tile([P, 512], F32, tag="ps2")
                    for kc in range(KC):
                        nc.tensor.matmul(
                            ps[:rows, :],
                            lhsT=xt2[:, kc, :rows],
                            rhs=wv_sb[:, kc, nv * 512:(nv + 1) * 512],
                            start=(kc == 0), stop=(kc == KC - 1))
                    src = ps[:rows, :].rearrange("p (h d) -> p h d", h=8)
                    if m < 18:
                        dst = v_sb[:rows, m, nv * 8:(nv + 1) * 8, 0:64]
                    else:
                        dst = vg_sb[:rows, nv * 8:(nv + 1) * 8, 0:64]
                    nc.vector.tensor_copy(dst, src)

        # ---------------- phase 2: attention ----------------
        with ExitStack() as s3:
            pool_probs = s3.enter_context(tc.tile_pool(name="probs", bufs=3))
            pool_pxg = s3.enter_context(tc.tile_pool(name="pxg", bufs=2))
            pool_ps_s = s3.enter_context(tc.tile_pool(name="ps_s", bufs=2, space="PSUM"))
            pool_ps_o = s3.enter_context(tc.tile_pool(name="ps_o", bufs=2, space="PSUM"))
            pool_ps_gx = s3.enter_context(tc.tile_pool(name="ps_gx", bufs=1, space="PSUM"))
            pool_ps_xg = s3.enter_context(tc.tile_pool(name="ps_xg", bufs=1, space="PSUM"))
            pool_nrm = s3.enter_context(tc.tile_pool(name="nrm", bufs=3))

            for h in range(H):
                hp, hb = h // 2, 64 * (h % 2)
                qk = lambda sb, c0, c1: sb[hb:hb + 64, hp, c0:c1]

                # xg scores (local q vs global k), k-major [16, 2048]
                pxg = pool_pxg.tile([G, T_OWN], ATT, tag="pxg")
                for nq in range(4):
                    psx = pool_ps_xg.tile([G, 512], F32, tag="psxg")
                    nc.tensor.matmul(psx[:, :],
                                     lhsT=qk(kT_sb, G0, G0 + G),
                                     rhs=qk(qT_sb, nq * 512, (nq + 1) * 512),
                                     start=True, stop=True)
                    nc.scalar.activation(pxg[:, nq * 512:(nq + 1) * 512],
                                         psx[:, :], Exp, scale=SCALE)

                ps_gx = pool_ps_gx.tile([DV + 1, G], F32, tag="psgx")
                probs = {}

                def do_pv(i):
                    ps_o = pool_ps_o.tile([P, 130], F32, tag="ps_o")
                    for dj, j in enumerate((i - 1, i, i + 1)):
                        pj, cb, jlo = probs[j]
                        c0 = cb + (i - jlo) * BLOCK
                        nc.tensor.matmul(ps_o[:, 0:65],
                                         lhsT=pj[:, c0:c0 + BLOCK],
                                         rhs=v_sb[:, _vblk(j), h, 0:65],
                                         start=(dj == 0), stop=(dj == 2))
                    nc.tensor.matmul(ps_o[:, 65:130],
                                     lhsT=pxg[:, i * BLOCK:(i + 1) * BLOCK],
                                     rhs=vg_sb[:, h, 0:65],
                                     start=True, stop=True)
                    rec = pool_nrm.tile([P, 2], F32, tag="rec")
                    dns = ps_o[:].rearrange("p (a b) -> p a b", a=2)[:, :, 64]
                    nc.vector.reciprocal(rec[:, 0:2], dns)
                    tG = pool_nrm.tile([P, DV], ATT, tag="tG")
                    nc.vector.tensor_scalar_mul(tG[:], ps_o[:, 65:129],
                                                rec[:, 1:2])
                    nc.vector.scalar_tensor_tensor(
                        out_x[:, i, h * DV:(h + 1) * DV],
                        ps_o[:, 0:64], rec[:, 0:1], tG[:],
                        op0=mybir.AluOpType.mult, op1=mybir.AluOpType.add)

                for rp in range(9):   # k-block pairs (-1,0), (1,2), ... (15,16)
                    ps_s = pool_ps_s.tile([P, 1024], F32, tag="ps_s")
                    pt = pool_probs.tile([P, 1024], ATT, tag="probs")
                    ntot_max = 0
                    for sub in range(2):
                        r_ = 2 * rp - 1 + sub
                        cb = 512 * sub
                        ilo, ihi = max(r_ - 1, 0), min(r_ + 1, QB - 1)
                        nloc = (ihi - ilo + 1) * BLOCK
                        own = 0 <= r_ <= 15
                        ntot = nloc + (G if own else 0)
                        ntot_max = cb + ntot
                        kc0 = _kcols(r_)
                        nc.tensor.matmul(ps_s[:, cb:cb + nloc],
                                         lhsT=qk(kT_sb, kc0, kc0 + BLOCK),
                                         rhs=qk(qT_sb, ilo * BLOCK,
                                                (ihi + 1) * BLOCK),
                                         start=True, stop=True)
                        if own:  # gx scores appended (global q vs this k-block)
                            nc.tensor.matmul(ps_s[:, cb + nloc:cb + ntot],
                                             lhsT=qk(kT_sb, kc0, kc0 + BLOCK),
                                             rhs=qk(qT_sb, QG0, QG0 + G),
                                             start=True, stop=True)
                        probs[r_] = (pt, cb, ilo)
                    nc.scalar.activation(pt[:, :ntot_max], ps_s[:, :ntot_max],
                                         Exp, scale=SCALE)
                    for sub in range(2):
                        r_ = 2 * rp - 1 + sub
                        if 0 <= r_ <= 15:   # gx numerator/denominator accum
                            _, cb, ilo = probs[r_]
                            ihi = min(r_ + 1, QB - 1)
                            nloc = (ihi - ilo + 1) * BLOCK
                            nc.tensor.matmul(ps_gx[:, :],
                                             lhsT=v_sb[:, r_, h, 0:65],
                                             rhs=pt[:, cb + nloc:cb + nloc + G],
                                             start=(r_ == 0), stop=(r_ == 15))
                    for sub in range(2):
                        i = 2 * rp - 2 + sub  # q-blocks whose windows completed
                        if 0 <= i <= QB - 1:
                            do_pv(i)
                    for rr in list(probs):
                        if rr < 2 * rp - 2:
                            probs.pop(rr)
                # stash gx partials for this head
                nc.vector.tensor_copy(gx_sb[:, h, :], ps_gx[:, :])

            nc.sync.dma_start(out=gx_part_d[:], in_=gx_sb[:])
            nc.gpsimd.collective_compute(
                "AllReduce", mybir.AluOpType.add,
                replica_groups=[[0, 1, 2, 3], [4, 5, 6, 7]],
                ins=[gx_part_d.opt()], outs=[gx_full_d.opt()])

        # ---------------- phase 3: output projection ----------------
        with ExitStack() as s4:
            pool_wo = s4.enter_context(tc.tile_pool(name="wo", bufs=1))
            pool_ot = s4.enter_context(tc.tile_pool(name="ot", bufs=10))
            pool_pst = s4.enter_context(tc.tile_pool(name="pst", bufs=2, space="PSUM"))
            pool_psy = s4.enter_context(tc.tile_pool(name="psy", bufs=2, space="PSUM"))
            pool_ysb = s4.enter_context(tc.tile_pool(name="ysb", bufs=3))
            pool_gxf = s4.enter_context(tc.tile_pool(name="gxf", bufs=1))
            wo_sb = pool_wo.tile([P, KC, D_MODEL], R32)
            bo_sb = pool_wo.tile([1, D_MODEL], F32)
            ones1 = pool_wo.tile([1, P], F32)
            bias_sb = pool_wo.tile([P, D_MODEL], F32)
            ident = pool_wo.tile([P, P], ATT)
            nc.sync.dma_start(out=wo_sb[:], in_=rb(woT_d[:]))
            nc.sync.dma_start(out=bo_sb[:], in_=bo_d[:])
            nc.vector.memset(ones1[:], 1.0)
            make_identity(nc, ident[:])
            for nv in range(2):
                psb0 = pool_psy.tile([P, 512], F32, tag="psy")
                nc.tensor.matmul(psb0[:], lhsT=ones1[:],
                                 rhs=bo_sb[:, nv * 512:(nv + 1) * 512],
                                 start=True, stop=True)
                nc.scalar.activation(bias_sb[:, nv * 512:(nv + 1) * 512],
                                     psb0[:], Copy)

            for m in range(QB):
                ots = []
                for kc in range(KC):
                    pst = pool_pst.tile([P, P], ATT, tag="pst")
                    nc.tensor.transpose(pst[:],
                                        out_x[:, m, kc * P:(kc + 1) * P],
                                        ident[:])
                    ot = pool_ot.tile([P, P], R32, tag="ot")
                    nc.vector.tensor_copy(ot[:], pst[:])
                    ots.append(ot)
                for nv in range(2):
                    psy = pool_psy.tile([P, 512], F32, tag="psy")
                    for kc in range(KC):
                        nc.tensor.matmul(psy[:],
                                         lhsT=ots[kc][:],
                                         rhs=wo_sb[:, kc, nv * 512:(nv + 1) * 512],
                                         start=(kc == 0), stop=(kc == KC - 1))
                    ysb = pool_ysb.tile([P, 512], F32, tag="ysb")
                    nc.vector.tensor_add(ysb[:], psy[:],
                                         bias_sb[:, nv * 512:(nv + 1) * 512])
                    nc.sync.dma_start(
                        out=y_own_d[m * P:(m + 1) * P, nv * 512:(nv + 1) * 512],
                        in_=ysb[:])

            # ----- global rows: normalize gx and project -----
            num_sb = pool_gxf.tile([P, KC, G], F32)     # [(h d) chunks, g]
            den_sb = pool_gxf.tile([H, G], F32)
            rden = pool_gxf.tile([H, G], F32)
            sel = pool_gxf.tile([H, H * 64], F32)
            norm_sb = pool_gxf.tile([P, KC, G], R32)
            nc.gpsimd.memset(sel[:], 0.0)
            sel3 = sel[:].rearrange("k (h d) -> k h d", h=H)
            nc.gpsimd.affine_select(
                out=sel3, in_=sel3,
                compare_op=mybir.AluOpType.not_equal, fill=1.0,
                base=0, pattern=[[-1, H], [0, 64]], channel_multiplier=1)
            for h in range(H):
                nc.sync.dma_start(
                    out=num_sb[64 * (h % 2):64 * (h % 2) + 64, h // 2, :],
                    in_=gx_full_d[0:64, h, :])
            nc.sync.dma_start(out=den_sb[:], in_=gx_full_d[64, :, :])
            nc.vector.reciprocal(rden[:], den_sb[:])
            for h in range(H):
                psb = pool_pst.tile([64, G], F32, tag="pst")
                nc.tensor.matmul(psb[:], lhsT=sel[:, h * 64:(h + 1) * 64],
                                 rhs=rden[:], start=True, stop=True)
                sl = (slice(64 * (h % 2), 64 * (h % 2) + 64), h // 2, slice(None))
                nc.vector.tensor_mul(norm_sb[sl], num_sb[sl], psb[:])
            for nv in range(2):
                psy = pool_psy.tile([G, 512], F32, tag="psy")
                for kc in range(KC):
                    nc.tensor.matmul(psy[:],
                                     lhsT=norm_sb[:, kc, :],
                                     rhs=wo_sb[:, kc, nv * 512:(nv + 1) * 512],
                                     start=(kc == 0), stop=(kc == KC - 1))
                ygsb = pool_ysb.tile([G, 512], F32, tag="ygsb")
                nc.vector.tensor_add(ygsb[:], psy[:],
                                     bias_sb[0:G, nv * 512:(nv + 1) * 512])
                nc.sync.dma_start(out=y_g_d[:, nv * 512:(nv + 1) * 512],
                                  in_=ygsb[:])

    nc.compile()
    return nc


def shard_inputs(x, Wq, Wk, Wv, Wo, bo):
    """Build the 8 per-core input maps."""
    x = np.asarray(x, dtype=np.float32)
    wqT = np.ascontiguousarray(
        np.asarray(Wq, np.float32).T.reshape(KC, P, H * DK).transpose(1, 0, 2))
    wkT = np.ascontiguousarray(
        np.asarray(Wk, np.float32).T.reshape(KC, P, H * DK).transpose(1, 0, 2))
    wvT = np.ascontiguousarray(
        np.asarray(Wv, np.float32).T.reshape(KC, P, H * DV).transpose(1, 0, 2))
    woT = np.ascontiguousarray(
        np.asarray(Wo, np.float32).T.reshape(KC, P, D_MODEL).transpose(1, 0, 2))
    bo2 = np.asarray(bo, np.float32).reshape(1, D_MODEL)
    in_maps = []
    for c in range(N_CORES):
        b, t = c // 4, c % 4
        xg = x[b, :G]                       # [16, 1024]
        xl = x[b, G:]                       # [8192, 1024]
        own = xl[t * T_OWN:(t + 1) * T_OWN]
        hl = xl[((16 * t - 1) % NBLK) * BLOCK:][:BLOCK]
        hr = xl[((16 * t + 16) % NBLK) * BLOCK:][:BLOCK]
        xc = np.concatenate([own, hl, hr, xg], axis=0)          # [2320, 1024]
        xT = np.ascontiguousarray(
            xc.T.reshape(KC, P, XC).transpose(1, 0, 2))         # [128, 8, 2320]
        in_maps.append({"xin": xT, "wqT": wqT, "wkT": wkT, "wvT": wvT,
                        "woT": woT, "bo": bo2})
    return in_maps


_NC_CACHE = {}


def get_program():
    key = (USE_F32R, ATT_BF16)
    if key not in _NC_CACHE:
        _NC_CACHE[key] = build_program()
    return _NC_CACHE[key]


def _install_ntff_hook():
    """Provide antenv.axon_hooks (missing in this image) so that
    run_bass_kernel_spmd(trace=True) can capture NTFF profiles."""
    import sys, types
    if "antenv.axon_hooks" in sys.modules:
        return
    try:
        import antenv  # noqa: F401
        from trn_agent_boot.trn_boot import _ntff_profile_via_ctypes
        mod = types.ModuleType("antenv.axon_hooks")
        mod._hook = _ntff_profile_via_ctypes("/opt/axon/libaxon_pjrt.so")
        mod.set_axon_ntff_profile_hook = lambda h: setattr(mod, "_hook", h)
        mod.get_axon_ntff_profile_hook = lambda: mod._hook
        sys.modules["antenv.axon_hooks"] = mod
    except Exception as e:  # profiling is optional
        print(f"ntff hook install failed: {e}")


def run(x, Wq, Wk, Wv, Wo, bo, trace=False):
    from concourse.bass_utils import run_bass_kernel_spmd
    if trace:
        _install_ntff_hook()
    nc = get_program()
    in_maps = shard_inputs(x, Wq, Wk, Wv, Wo, bo)
    res = run_bass_kernel_spmd(nc, in_maps, list(range(N_CORES)), trace=trace)
    y = np.empty((B, T, D_MODEL), dtype=np.float32)
    for c in range(N_CORES):
        b, t = c // 4, c % 4
        if t == 0:
            y[b, :G] = res.results[c]["y_g"]
        y[b, G + t * T_OWN:G + (t + 1) * T_OWN] = res.results[c]["y_own"]
    return y, res


def kernel(x, Wq, Wk, Wv, Wo, bo):
    y, _ = run(x, Wq, Wk, Wv, Wo, bo, trace=False)
    return y


# revision 14
# speedup vs baseline: 1.1384x; 1.0347x over previous
"""BigBird attention Trainium2 kernel (Bass/Tile), 8-core SPMD.

Sharding: core c -> (batch b = c//4, sequence quarter t = c%4).
Each core computes ALL 16 heads for its 2048 "own" local tokens, plus a
1-block (128 token) halo on each side (recomputed locally, circular) and
the 16 global tokens.  Outputs are disjoint rows of y, so the host gather
is pure concatenation.  The only cross-core communication is a 66 KB
AllReduce of the global-query attention partial sums (numerator+denominator).

Device x column layout per core (2320 cols): [own 2048 | hl 128 | hr 128 | g 16].
"""

import os
import numpy as np

# ---------------- problem constants (hardcoded per contract) ----------------
D_MODEL = 1024
H = 16
DK = 64
DV = 64
BLOCK = 128
G = 16
B = 2
T = G + 8192          # 8208
NBLK = 64             # local blocks per batch
QB = 16               # own q blocks per core
T_OWN = QB * BLOCK    # 2048
XC = T_OWN + 2 * BLOCK + G  # 2320 device x cols: [own | hl | hr | g]
N_CORES = 8
P = 128
KC = D_MODEL // P     # 8 contraction chunks
MC = (H * DK) // P    # 8 row chunks of qT/kT (2 heads per chunk)
SCALE = 1.0 / 8.0     # 1/sqrt(64)

# dtype knobs
USE_F32R = os.environ.get("BB_NO_F32R", "") == ""     # fp32r matmuls for fp32 data
ATT_BF16 = os.environ.get("BB_ATT_F32", "") == ""     # bf16 q/k/v/probs/out_x storage

# column offsets in the device-x layout
OWN0 = 0
HL0 = T_OWN            # 2048
HR0 = T_OWN + BLOCK    # 2176
G0 = T_OWN + 2 * BLOCK # 2304 (globals in kT / x layout)
QXC = T_OWN + G        # 2064 qT cols: [own | g]
QG0 = T_OWN            # globals offset within qT


def _kcols(r):
    """Columns of k-block with relative index r in [-1, 16]."""
    if r == -1:
        return HL0
    if r == 16:
        return HR0
    return r * BLOCK


def _vblk(r):
    """v_sb block index for relative k-block r."""
    if r == -1:
        return 16
    if r == 16:
        return 17
    return r


def build_program():
    import concourse.bacc as bacc
    import concourse.tile as tile
    import concourse.mybir as mybir
    from concourse.masks import make_identity
    from contextlib import ExitStack

    dt = mybir.dt
    F32 = dt.float32
    ATT = dt.bfloat16 if ATT_BF16 else dt.float32
    MMDT = dt.float32r if USE_F32R else dt.float32
    R32 = MMDT
    Exp = mybir.ActivationFunctionType.Exp
    Copy = mybir.ActivationFunctionType.Copy

    nc = bacc.Bacc("TRN2", target_bir_lowering=False, debug=False,
                   num_devices=N_CORES)

    def rb(ap):  # bitcast an fp32 AP (e.g. DRAM input view) to float32r
        return ap.bitcast(R32) if USE_F32R else ap

    # ---------------- external I/O ----------------
    xT_d = nc.dram_tensor("xin", [P, KC, XC], F32, kind="ExternalInput").ap()
    wqT_d = nc.dram_tensor("wqT", [P, KC, H * DK], F32, kind="ExternalInput").ap()
    wkT_d = nc.dram_tensor("wkT", [P, KC, H * DK], F32, kind="ExternalInput").ap()
    wvT_d = nc.dram_tensor("wvT", [P, KC, H * DV], F32, kind="ExternalInput").ap()
    woT_d = nc.dram_tensor("woT", [P, KC, D_MODEL], F32, kind="ExternalInput").ap()
    bo_d = nc.dram_tensor("bo", [1, D_MODEL], F32, kind="ExternalInput").ap()
    y_own_d = nc.dram_tensor("y_own", [T_OWN, D_MODEL], F32,
                             kind="ExternalOutput").ap()
    y_g_d = nc.dram_tensor("y_g", [G, D_MODEL], F32, kind="ExternalOutput").ap()

    with tile.TileContext(nc) as tc, ExitStack() as top:
        # ------------- persistent SBUF -------------
        pool_qT = top.enter_context(tc.tile_pool(name="qT", bufs=1))
        pool_kT = top.enter_context(tc.tile_pool(name="kT", bufs=1))
        pool_v = top.enter_context(tc.tile_pool(name="v", bufs=1))
        pool_misc = top.enter_context(tc.tile_pool(name="misc", bufs=1))
        pool_outx = top.enter_context(tc.tile_pool(name="outx", bufs=1))
        out_x = pool_outx.tile([P, QB, H * DV], ATT)
        qT_sb = pool_qT.tile([P, MC, QXC], ATT)       # rows (h,d) chunked, cols t
        kT_sb = pool_kT.tile([P, MC, XC], ATT)
        v_sb = pool_v.tile([P, 18, H, 65], ATT)      # [row%128, kblk, h, d(+1)]
        vg_sb = pool_misc.tile([G, H, 65], ATT)      # global v rows
        gx_sb = pool_misc.tile([DV + 1, H, G], F32)  # gx partials [d(+den), h, g]
        nc.gpsimd.memset(v_sb[:, :, :, 64:65], 1.0)
        nc.gpsimd.memset(vg_sb[:, :, 64:65], 1.0)

        # DRAM bounce buffers for the gx AllReduce
        pool_dram = top.enter_context(tc.tile_pool(name="dram", bufs=1, space="DRAM"))
        gx_part_d = pool_dram.tile([DV + 1, H, G], F32)
        gx_full_d = pool_dram.tile([DV + 1, H, G], F32)

        # ---------------- phase 1a: q,k projections ----------------
        NW = 512

        def _segments(pairs):
            """Split (src0, dst0, width) pairs into <=NW chunks."""
            out = []
            for src0, dst0, width in pairs:
                o = 0
                while o < width:
                    w = min(NW, width - o)
                    out.append((src0 + o, dst0 + o, w))
                    o += w
            return out

        q_segs = _segments([(OWN0, 0, T_OWN), (G0, QG0, G)])
        k_segs = _segments([(0, 0, XC)])
        for pname, w_d, dst, segs, use_act in (("q", wqT_d, qT_sb, q_segs, True),
                                               ("k", wkT_d, kT_sb, k_segs, False)):
            with ExitStack() as s1:
                pool_w1 = s1.enter_context(tc.tile_pool(name=f"w1{pname}", bufs=1))
                pool_x1 = s1.enter_context(tc.tile_pool(name=f"x1{pname}", bufs=2))
                pool_ps1 = s1.enter_context(
                    tc.tile_pool(name=f"ps1{pname}", bufs=4, space="PSUM"))
                w_sb = pool_w1.tile([P, KC, H * DK], R32, name=f"w_{pname}")
                nc.sync.dma_start(out=w_sb[:], in_=rb(w_d[:]))
                for src0, dst0, nw in segs:
                    xt = pool_x1.tile([P, KC, NW], R32, tag="xt", name="xt")
                    nc.sync.dma_start(out=xt[:, :, :nw],
                                      in_=rb(xT_d[:, :, src0:src0 + nw]))
                    for mc in range(MC):
                        ps = pool_ps1.tile([P, NW], F32, tag="ps1", name="ps")
                        for kc in range(KC):
                            nc.tensor.matmul(
                                ps[:, :nw],
                                lhsT=w_sb[:, kc, mc * P:(mc + 1) * P],
                                rhs=xt[:, kc, :nw],
                                start=(kc == 0), stop=(kc == KC - 1))
                        if use_act:
                            nc.scalar.activation(dst[:, mc, dst0:dst0 + nw],
                                                 ps[:, :nw], Copy)
                        else:
                            nc.vector.tensor_copy(dst[:, mc, dst0:dst0 + nw],
                                                  ps[:, :nw])

        # ---------------- phase 1b: v projection ----------------
        with ExitStack() as s2:
            pool_w2 = s2.enter_context(tc.tile_pool(name="w2", bufs=1))
            pool_x2 = s2.enter_context(tc.tile_pool(name="x2", bufs=3))
            pool_ps2 = s2.enter_context(tc.tile_pool(name="ps2", bufs=3, space="PSUM"))
            wv_sb = pool_w2.tile([P, KC, H * DV], R32)
            nc.sync.dma_start(out=wv_sb[:], in_=rb(wvT_d[:]))
            for m in range(19):           # 18 local blocks + globals(16 rows)
                rows = P if m < 18 else G
                xt2 = pool_x2.tile([P, KC, P], R32, tag="xt2")
                nc.sync.dma_start(out=xt2[:, :, :rows],
                                  in_=rb(xT_d[:, :, m * P:m * P + rows]))
                for nv in range(2):       # v inner-dim halves (8 heads each)
                    ps = pool_ps2.tile([P, 512], F32, tag="ps2")
                    for kc in range(KC):
                        nc.tensor.matmul(
                            ps[:rows, :],
                            lhsT=xt2[:, kc, :rows],
                            rhs=wv_sb[:, kc, nv * 512:(nv + 1) * 512],
                            start=(kc == 0), stop=(kc == KC - 1))
                    src = ps[:rows, :].rearrange("p (h d) -> p h d", h=8)
                    if m < 18:
                        dst = v_sb[:rows, m, nv * 8:(nv + 1) * 8, 0:64]
                    else:
                        dst = vg_sb[:rows, nv * 8:(nv + 1) * 8, 0:64]
                    nc.vector.tensor_copy(dst, src)

        # ---------------- phase 2: attention ----------------
        with ExitStack() as s3:
            pool_probs = s3.enter_context(tc.tile_pool(name="probs", bufs=3))
            pool_pxg = s3.enter_context(tc.tile_pool(name="pxg", bufs=2))
            pool_ps_s = s3.enter_context(tc.tile_pool(name="ps_s", bufs=2, space="PSUM"))
            pool_ps_o = s3.enter_context(tc.tile_pool(name="ps_o", bufs=2, space="PSUM"))
            pool_ps_gx = s3.enter_context(tc.tile_pool(name="ps_gx", bufs=1, space="PSUM"))
            pool_ps_xg = s3.enter_context(tc.tile_pool(name="ps_xg", bufs=1, space="PSUM"))
            pool_nrm = s3.enter_context(tc.tile_pool(name="nrm", bufs=3))

            for h in range(H):
                hp, hb = h // 2, 64 * (h % 2)
                qk = lambda sb, c0, c1: sb[hb:hb + 64, hp, c0:c1]

                # xg scores (local q vs global k), k-major [16, 2048]
                pxg = pool_pxg.tile([G, T_OWN], ATT, tag="pxg")
                for nq in range(4):
                    psx = pool_ps_xg.tile([G, 512], F32, tag="psxg")
                    nc.tensor.matmul(psx[:, :],
                                     lhsT=qk(kT_sb, G0, G0 + G),
                                     rhs=qk(qT_sb, nq * 512, (nq + 1) * 512),
                                     start=True, stop=True)
                    nc.scalar.activation(pxg[:, nq * 512:(nq + 1) * 512],
                                         psx[:, :], Exp, scale=SCALE)

                ps_gx = pool_ps_gx.tile([DV + 1, G], F32, tag="psgx")
                probs = {}

                def do_pv(i):
                    ps_o = pool_ps_o.tile([P, 130], F32, tag="ps_o")
                    for dj, j in enumerate((i - 1, i, i + 1)):
                        pj, cb, jlo = probs[j]
                        c0 = cb + (i - jlo) * BLOCK
                        nc.tensor.matmul(ps_o[:, 0:65],
                                         lhsT=pj[:, c0:c0 + BLOCK],
                                         rhs=v_sb[:, _vblk(j), h, 0:65],
                                         start=(dj == 0), stop=(dj == 2))
                    nc.tensor.matmul(ps_o[:, 65:130],
                                     lhsT=pxg[:, i * BLOCK:(i + 1) * BLOCK],
                                     rhs=vg_sb[:, h, 0:65],
                                     start=True, stop=True)
                    rec = pool_nrm.tile([P, 2], F32, tag="rec")
                    dns = ps_o[:].rearrange("p (a b) -> p a b", a=2)[:, :, 64]
                    nc.vector.reciprocal(rec[:, 0:2], dns)
                    tG = pool_nrm.tile([P, DV], ATT, tag="tG")
                    nc.vector.tensor_scalar_mul(tG[:], ps_o[:, 65:129],
                                                rec[:, 1:2])
                    nc.vector.scalar_tensor_tensor(
                        out_x[:, i, h * DV:(h + 1) * DV],
                        ps_o[:, 0:64], rec[:, 0:1], tG[:],
                        op0=mybir.AluOpType.mult, op1=mybir.AluOpType.add)

                for rp in range(9):   # k-block pairs (-1,0), (1,2), ... (15,16)
                    ps_s = pool_ps_s.tile([P, 1024], F32, tag="ps_s")
                    pt = pool_probs.tile([P, 1024], ATT, tag="probs")
                    ntot_max = 0
                    for sub in range(2):
                        r_ = 2 * rp - 1 + sub
                        cb = 512 * sub
                        ilo, ihi = max(r_ - 1, 0), min(r_ + 1, QB - 1)
                        nloc = (ihi - ilo + 1) * BLOCK
                        own = 0 <= r_ <= 15
                        ntot = nloc + (G if own else 0)
                        ntot_max = cb + ntot
                        kc0 = _kcols(r_)
                        nc.tensor.matmul(ps_s[:, cb:cb + nloc],
                                         lhsT=qk(kT_sb, kc0, kc0 + BLOCK),
                                         rhs=qk(qT_sb, ilo * BLOCK,
                                                (ihi + 1) * BLOCK),
                                         start=True, stop=True)
                        if own:  # gx scores appended (global q vs this k-block)
                            nc.tensor.matmul(ps_s[:, cb + nloc:cb + ntot],
                                             lhsT=qk(kT_sb, kc0, kc0 + BLOCK),
                                             rhs=qk(qT_sb, QG0, QG0 + G),
                                             start=True, stop=True)
                        probs[r_] = (pt, cb, ilo)
                    nc.scalar.activation(pt[:, :ntot_max], ps_s[:, :ntot_max],
                                         Exp, scale=SCALE)
                    for sub in range(2):
                        r_ = 2 * rp - 1 + sub
                        if 0 <= r_ <= 15:   # gx numerator/denominator accum
                            _, cb, ilo = probs[r_]
                            ihi = min(r_ + 1, QB - 1)
                            nloc = (ihi - ilo + 1) * BLOCK
                            nc.tensor.matmul(ps_gx[:, :],
                                             lhsT=v_sb[:, r_, h, 0:65],
                                             rhs=pt[:, cb + nloc:cb + nloc + G],
                                             start=(r_ == 0), stop=(r_ == 15))
                    for sub in range(2):
                        i = 2 * rp - 2 + sub  # q-blocks whose windows completed
                        if 0 <= i <= QB - 1:
                            do_pv(i)
                    for rr in list(probs):
                        if rr < 2 * rp - 2:
                            probs.pop(rr)
                # stash gx partials for this head
                nc.vector.tensor_copy(gx_sb[:, h, :], ps_gx[:, :])

            nc.sync.dma_start(out=gx_part_d[:], in_=gx_sb[:])
            nc.gpsimd.collective_compute(
                "AllReduce", mybir.AluOpType.add,
                replica_groups=[[0, 1, 2, 3], [4, 5, 6, 7]],
                ins=[gx_part_d.opt()], outs=[gx_full_d.opt()])

        # ---------------- phase 3: output projection ----------------
        with ExitStack() as s4:
            pool_wo = s4.enter_context(tc.tile_pool(name="wo", bufs=1))
            pool_ot = s4.enter_context(tc.tile_pool(name="ot", bufs=10))
            pool_pst = s4.enter_context(tc.tile_pool(name="pst", bufs=3, space="PSUM"))
            pool_psy = s4.enter_context(tc.tile_pool(name="psy", bufs=2, space="PSUM"))
            pool_ysb = s4.enter_context(tc.tile_pool(name="ysb", bufs=3))
            pool_gxf = s4.enter_context(tc.tile_pool(name="gxf", bufs=1))
            wo_sb = pool_wo.tile([P, KC, D_MODEL], R32)
            bo_sb = pool_wo.tile([1, D_MODEL], F32)
            ones1 = pool_wo.tile([1, P], F32)
            bias_sb = pool_wo.tile([P, D_MODEL], F32)
            ident = pool_wo.tile([P, P], ATT)
            nc.sync.dma_start(out=wo_sb[:], in_=rb(woT_d[:]))
            nc.sync.dma_start(out=bo_sb[:], in_=bo_d[:])
            nc.vector.memset(ones1[:], 1.0)
            make_identity(nc, ident[:])
            for nv in range(2):
                psb0 = pool_psy.tile([P, 512], F32, tag="psy")
                nc.tensor.matmul(psb0[:], lhsT=ones1[:],
                                 rhs=bo_sb[:, nv * 512:(nv + 1) * 512],
                                 start=True, stop=True)
                nc.scalar.activation(bias_sb[:, nv * 512:(nv + 1) * 512],
                                     psb0[:], Copy)

            for m in range(QB):
                ots = []
                for kc in range(KC):
                    pst = pool_pst.tile([P, P], ATT, tag="pst")
                    nc.tensor.transpose(pst[:],
                                        out_x[:, m, kc * P:(kc + 1) * P],
                                        ident[:])
                    ot = pool_ot.tile([P, P], R32, tag="ot")
                    nc.scalar.activation(ot[:], pst[:], Copy)
                    ots.append(ot)
                for nv in range(2):
                    psy = pool_psy.tile([P, 512], F32, tag="psy")
                    for kc in range(KC):
                        nc.tensor.matmul(psy[:],
                                         lhsT=ots[kc][:],
                                         rhs=wo_sb[:, kc, nv * 512:(nv + 1) * 512],
                                         start=(kc == 0), stop=(kc == KC - 1))
                    ysb = pool_ysb.tile([P, 512], F32, tag="ysb")
                    nc.vector.tensor_add(ysb[:], psy[:],
                                         bias_sb[:, nv * 512:(nv + 1) * 512])
                    nc.sync.dma_start(
                        out=y_own_d[m * P:(m + 1) * P, nv * 512:(nv + 1) * 512],
                        in_=ysb[:])

            # ----- global rows: normalize gx and project -----
            num_sb = pool_gxf.tile([P, KC, G], F32)     # [(h d) chunks, g]
            den_sb = pool_gxf.tile([H, G], F32)
            rden = pool_gxf.tile([H, G], F32)
            sel = pool_gxf.tile([H, H * 64], F32)
            norm_sb = pool_gxf.tile([P, KC, G], R32)
            nc.gpsimd.memset(sel[:], 0.0)
            sel3 = sel[:].rearrange("k (h d) -> k h d", h=H)
            nc.gpsimd.affine_select(
                out=sel3, in_=sel3,
                compare_op=mybir.AluOpType.not_equal, fill=1.0,
                base=0, pattern=[[-1, H], [0, 64]], channel_multiplier=1)
            for h in range(H):
                nc.sync.dma_start(
                    out=num_sb[64 * (h % 2):64 * (h % 2) + 64, h // 2, :],
                    in_=gx_full_d[0:64, h, :])
            nc.sync.dma_start(out=den_sb[:], in_=gx_full_d[64, :, :])
            nc.vector.reciprocal(rden[:], den_sb[:])
            for h in range(H):
                psb = pool_pst.tile([64, G], F32, tag="pst")
                nc.tensor.matmul(psb[:], lhsT=sel[:, h * 64:(h + 1) * 64],
                                 rhs=rden[:], start=True, stop=True)
                sl = (slice(64 * (h % 2), 64 * (h % 2) + 64), h // 2, slice(None))
                nc.vector.tensor_mul(norm_sb[sl], num_sb[sl], psb[:])
            for nv in range(2):
                psy = pool_psy.tile([G, 512], F32, tag="psy")
                for kc in range(KC):
                    nc.tensor.matmul(psy[:],
                                     lhsT=norm_sb[:, kc, :],
                                     rhs=wo_sb[:, kc, nv * 512:(nv + 1) * 512],
                                     start=(kc == 0), stop=(kc == KC - 1))
                ygsb = pool_ysb.tile([G, 512], F32, tag="ygsb")
                nc.vector.tensor_add(ygsb[:], psy[:],
                                     bias_sb[0:G, nv * 512:(nv + 1) * 512])
                nc.sync.dma_start(out=y_g_d[:, nv * 512:(nv + 1) * 512],
                                  in_=ygsb[:])

    nc.compile()
    return nc


def shard_inputs(x, Wq, Wk, Wv, Wo, bo):
    """Build the 8 per-core input maps."""
    x = np.asarray(x, dtype=np.float32)
    wqT = np.ascontiguousarray(
        np.asarray(Wq, np.float32).T.reshape(KC, P, H * DK).transpose(1, 0, 2))
    wkT = np.ascontiguousarray(
        np.asarray(Wk, np.float32).T.reshape(KC, P, H * DK).transpose(1, 0, 2))
    wvT = np.ascontiguousarray(
        np.asarray(Wv, np.float32).T.reshape(KC, P, H * DV).transpose(1, 0, 2))
    woT = np.ascontiguousarray(
        np.asarray(Wo, np.float32).T.reshape(KC, P, D_MODEL).transpose(1, 0, 2))
    bo2 = np.asarray(bo, np.float32).reshape(1, D_MODEL)
    in_maps = []
    for c in range(N_CORES):
        b, t = c // 4, c % 4
        xg = x[b, :G]                       # [16, 1024]
        xl = x[b, G:]                       # [8192, 1024]
        own = xl[t * T_OWN:(t + 1) * T_OWN]
        hl = xl[((16 * t - 1) % NBLK) * BLOCK:][:BLOCK]
        hr = xl[((16 * t + 16) % NBLK) * BLOCK:][:BLOCK]
        xc = np.concatenate([own, hl, hr, xg], axis=0)          # [2320, 1024]
        xT = np.ascontiguousarray(
            xc.T.reshape(KC, P, XC).transpose(1, 0, 2))         # [128, 8, 2320]
        in_maps.append({"xin": xT, "wqT": wqT, "wkT": wkT, "wvT": wvT,
                        "woT": woT, "bo": bo2})
    return in_maps


_NC_CACHE = {}


def get_program():
    key = (USE_F32R, ATT_BF16)
    if key not in _NC_CACHE:
        _NC_CACHE[key] = build_program()
    return _NC_CACHE[key]


def _install_ntff_hook():
    """Provide antenv.axon_hooks (missing in this image) so that
    run_bass_kernel_spmd(trace=True) can capture NTFF profiles."""
    import sys, types
    if "antenv.axon_hooks" in sys.modules:
        return
    try:
        import antenv  # noqa: F401
        from trn_agent_boot.trn_boot import _ntff_profile_via_ctypes
        mod = types.ModuleType("antenv.axon_hooks")
        mod._hook = _ntff_profile_via_ctypes("/opt/axon/libaxon_pjrt.so")
        mod.set_axon_ntff_profile_hook = lambda h: setattr(mod, "_hook", h)
        mod.get_axon_ntff_profile_hook = lambda: mod._hook
        sys.modules["antenv.axon_hooks"] = mod
    except Exception as e:  # profiling is optional
        print(f"ntff hook install failed: {e}")


def run(x, Wq, Wk, Wv, Wo, bo, trace=False):
    from concourse.bass_utils import run_bass_kernel_spmd
    if trace:
        _install_ntff_hook()
    nc = get_program()
    in_maps = shard_inputs(x, Wq, Wk, Wv, Wo, bo)
    res = run_bass_kernel_spmd(nc, in_maps, list(range(N_CORES)), trace=trace)
    y = np.empty((B, T, D_MODEL), dtype=np.float32)
    for c in range(N_CORES):
        b, t = c // 4, c % 4
        if t == 0:
            y[b, :G] = res.results[c]["y_g"]
        y[b, G + t * T_OWN:G + (t + 1) * T_OWN] = res.results[c]["y_own"]
    return y, res


def kernel(x, Wq, Wk, Wv, Wo, bo):
    y, _ = run(x, Wq, Wk, Wv, Wo, bo, trace=False)
    return y


# revision 15
# speedup vs baseline: 1.1800x; 1.0365x over previous
"""BigBird attention Trainium2 kernel (Bass/Tile), 8-core SPMD.

Sharding: core c -> (batch b = c//4, sequence quarter t = c%4).
Each core computes ALL 16 heads for its 2048 "own" local tokens, plus a
1-block (128 token) halo on each side (recomputed locally, circular) and
the 16 global tokens.  Outputs are disjoint rows of y, so the host gather
is pure concatenation.  The only cross-core communication is a 66 KB
AllReduce of the global-query attention partial sums (numerator+denominator).

Device x column layout per core (2320 cols): [own 2048 | hl 128 | hr 128 | g 16].
"""

import os
import numpy as np

# ---------------- problem constants (hardcoded per contract) ----------------
D_MODEL = 1024
H = 16
DK = 64
DV = 64
BLOCK = 128
G = 16
B = 2
T = G + 8192          # 8208
NBLK = 64             # local blocks per batch
QB = 16               # own q blocks per core
T_OWN = QB * BLOCK    # 2048
XC = T_OWN + 2 * BLOCK + G  # 2320 device x cols: [own | hl | hr | g]
N_CORES = 8
P = 128
KC = D_MODEL // P     # 8 contraction chunks
MC = (H * DK) // P    # 8 row chunks of qT/kT (2 heads per chunk)
SCALE = 1.0 / 8.0     # 1/sqrt(64)

# dtype knobs
USE_F32R = os.environ.get("BB_NO_F32R", "") == ""     # fp32r matmuls for fp32 data
ATT_BF16 = os.environ.get("BB_ATT_F32", "") == ""     # bf16 q/k/v/probs/out_x storage

# column offsets in the device-x layout
OWN0 = 0
HL0 = T_OWN            # 2048
HR0 = T_OWN + BLOCK    # 2176
G0 = T_OWN + 2 * BLOCK # 2304 (globals in kT / x layout)
QXC = T_OWN + G        # 2064 qT cols: [own | g]
QG0 = T_OWN            # globals offset within qT


def _kcols(r):
    """Columns of k-block with relative index r in [-1, 16]."""
    if r == -1:
        return HL0
    if r == 16:
        return HR0
    return r * BLOCK


def _vblk(r):
    """v_sb block index for relative k-block r."""
    if r == -1:
        return 16
    if r == 16:
        return 17
    return r


def build_program():
    import concourse.bacc as bacc
    import concourse.tile as tile
    import concourse.mybir as mybir
    from concourse.masks import make_identity
    from contextlib import ExitStack

    dt = mybir.dt
    F32 = dt.float32
    ATT = dt.bfloat16 if ATT_BF16 else dt.float32
    MMDT = dt.float32r if USE_F32R else dt.float32
    R32 = MMDT
    Exp = mybir.ActivationFunctionType.Exp
    Copy = mybir.ActivationFunctionType.Copy

    nc = bacc.Bacc("TRN2", target_bir_lowering=False, debug=False,
                   num_devices=N_CORES)

    def rb(ap):  # bitcast an fp32 AP (e.g. DRAM input view) to float32r
        return ap.bitcast(R32) if USE_F32R else ap

    # ---------------- external I/O ----------------
    xT_d = nc.dram_tensor("xin", [P, KC, XC], F32, kind="ExternalInput").ap()
    wqT_d = nc.dram_tensor("wqT", [P, KC, H * DK], F32, kind="ExternalInput").ap()
    wkT_d = nc.dram_tensor("wkT", [P, KC, H * DK], F32, kind="ExternalInput").ap()
    wvT_d = nc.dram_tensor("wvT", [P, KC, H * DV], F32, kind="ExternalInput").ap()
    woT_d = nc.dram_tensor("woT", [P, KC, D_MODEL], F32, kind="ExternalInput").ap()
    bo_d = nc.dram_tensor("bo", [1, D_MODEL], F32, kind="ExternalInput").ap()
    y_own_d = nc.dram_tensor("y_own", [T_OWN, D_MODEL], F32,
                             kind="ExternalOutput").ap()
    y_g_d = nc.dram_tensor("y_g", [G, D_MODEL], F32, kind="ExternalOutput").ap()

    with tile.TileContext(nc) as tc, ExitStack() as top:
        # ------------- persistent SBUF -------------
        pool_qT = top.enter_context(tc.tile_pool(name="qT", bufs=1))
        pool_kT = top.enter_context(tc.tile_pool(name="kT", bufs=1))
        pool_v = top.enter_context(tc.tile_pool(name="v", bufs=1))
        pool_misc = top.enter_context(tc.tile_pool(name="misc", bufs=1))
        pool_outx = top.enter_context(tc.tile_pool(name="outx", bufs=1))
        out_x = pool_outx.tile([P, QB, H * DV], ATT)
        qT_sb = pool_qT.tile([P, MC, QXC], ATT)       # rows (h,d) chunked, cols t
        kT_sb = pool_kT.tile([P, MC, XC], ATT)
        v_sb = pool_v.tile([P, 18, H, 65], ATT)      # [row%128, kblk, h, d(+1)]
        vg_sb = pool_misc.tile([G, H, 65], ATT)      # global v rows
        gx_sb = pool_misc.tile([DV + 1, H, G], F32)  # gx partials [d(+den), h, g]
        nc.gpsimd.memset(v_sb[:, :, :, 64:65], 1.0)
        nc.gpsimd.memset(vg_sb[:, :, 64:65], 1.0)

        # DRAM bounce buffers for the gx AllReduce
        pool_dram = top.enter_context(tc.tile_pool(name="dram", bufs=1, space="DRAM"))
        gx_part_d = pool_dram.tile([DV + 1, H, G], F32)
        gx_full_d = pool_dram.tile([DV + 1, H, G], F32)

        # ---------------- phase 1a: q,k projections ----------------
        NW = 512

        def _segments(pairs):
            """Split (src0, dst0, width) pairs into <=NW chunks."""
            out = []
            for src0, dst0, width in pairs:
                o = 0
                while o < width:
                    w = min(NW, width - o)
                    out.append((src0 + o, dst0 + o, w))
                    o += w
            return out

        q_segs = _segments([(OWN0, 0, T_OWN), (G0, QG0, G)])
        k_segs = _segments([(0, 0, XC)])
        for pname, w_d, dst, segs, use_act in (("q", wqT_d, qT_sb, q_segs, True),
                                               ("k", wkT_d, kT_sb, k_segs, False)):
            with ExitStack() as s1:
                pool_w1 = s1.enter_context(tc.tile_pool(name=f"w1{pname}", bufs=1))
                pool_x1 = s1.enter_context(tc.tile_pool(name=f"x1{pname}", bufs=2))
                pool_ps1 = s1.enter_context(
                    tc.tile_pool(name=f"ps1{pname}", bufs=4, space="PSUM"))
                w_sb = pool_w1.tile([P, KC, H * DK], R32, name=f"w_{pname}")
                nc.sync.dma_start(out=w_sb[:], in_=rb(w_d[:]))
                for src0, dst0, nw in segs:
                    xt = pool_x1.tile([P, KC, NW], R32, tag="xt", name="xt")
                    nc.sync.dma_start(out=xt[:, :, :nw],
                                      in_=rb(xT_d[:, :, src0:src0 + nw]))
                    for mc in range(MC):
                        ps = pool_ps1.tile([P, NW], F32, tag="ps1", name="ps")
                        for kc in range(KC):
                            nc.tensor.matmul(
                                ps[:, :nw],
                                lhsT=w_sb[:, kc, mc * P:(mc + 1) * P],
                                rhs=xt[:, kc, :nw],
                                start=(kc == 0), stop=(kc == KC - 1))
                        if use_act:
                            nc.scalar.activation(dst[:, mc, dst0:dst0 + nw],
                                                 ps[:, :nw], Copy)
                        else:
                            nc.vector.tensor_copy(dst[:, mc, dst0:dst0 + nw],
                                                  ps[:, :nw])

        # ---------------- phase 1b: v projection ----------------
        with ExitStack() as s2:
            pool_w2 = s2.enter_context(tc.tile_pool(name="w2", bufs=1))
            pool_x2 = s2.enter_context(tc.tile_pool(name="x2", bufs=4))
            pool_ps2 = s2.enter_context(tc.tile_pool(name="ps2", bufs=4, space="PSUM"))
            wv_sb = pool_w2.tile([P, KC, H * DV], R32)
            nc.sync.dma_start(out=wv_sb[:], in_=rb(wvT_d[:]))
            for m in range(19):           # 18 local blocks + globals(16 rows)
                rows = P if m < 18 else G
                xt2 = pool_x2.tile([P, KC, P], R32, tag="xt2")
                nc.sync.dma_start(out=xt2[:, :, :rows],
                                  in_=rb(xT_d[:, :, m * P:m * P + rows]))
                for nv in range(2):       # v inner-dim halves (8 heads each)
                    ps = pool_ps2.tile([P, 512], F32, tag="ps2")
                    for kc in range(KC):
                        nc.tensor.matmul(
                            ps[:rows, :],
                            lhsT=xt2[:, kc, :rows],
                            rhs=wv_sb[:, kc, nv * 512:(nv + 1) * 512],
                            start=(kc == 0), stop=(kc == KC - 1))
                    src = ps[:rows, :].rearrange("p (h d) -> p h d", h=8)
                    if m < 18:
                        dst = v_sb[:rows, m, nv * 8:(nv + 1) * 8, 0:64]
                    else:
                        dst = vg_sb[:rows, nv * 8:(nv + 1) * 8, 0:64]
                    nc.vector.tensor_copy(dst, src)

        # ---------------- phase 2: attention ----------------
        with ExitStack() as s3:
            pool_probs = s3.enter_context(tc.tile_pool(name="probs", bufs=4))
            pool_pxg = s3.enter_context(tc.tile_pool(name="pxg", bufs=2))
            pool_ps_s = s3.enter_context(tc.tile_pool(name="ps_s", bufs=2, space="PSUM"))
            pool_ps_o = s3.enter_context(tc.tile_pool(name="ps_o", bufs=2, space="PSUM"))
            pool_ps_gx = s3.enter_context(tc.tile_pool(name="ps_gx", bufs=1, space="PSUM"))
            pool_ps_xg = s3.enter_context(tc.tile_pool(name="ps_xg", bufs=1, space="PSUM"))
            pool_nrm = s3.enter_context(tc.tile_pool(name="nrm", bufs=3))

            for hp2 in range(H // 2):    # head pairs (2*hp2, 2*hp2+1)
                heads = (2 * hp2, 2 * hp2 + 1)
                hpch = hp2                   # qT/kT chunk index for this pair
                def qk(sb, h, c0, c1):
                    hb = 64 * (h % 2)
                    return sb[hb:hb + 64, hpch, c0:c1]

                # xg scores (local q vs global k), k-major [16, 2048] per head
                pxgs = {}
                for h in heads:
                    pxg = pool_pxg.tile([G, T_OWN], ATT, tag=f"pxg{h % 2}",
                                        name="pxg")
                    for nq in range(4):
                        psx = pool_ps_xg.tile([G, 512], F32, tag="psxg",
                                              name="psx")
                        nc.tensor.matmul(psx[:, :],
                                         lhsT=qk(kT_sb, h, G0, G0 + G),
                                         rhs=qk(qT_sb, h, nq * 512,
                                                (nq + 1) * 512),
                                         start=True, stop=True)
                        nc.scalar.activation(pxg[:, nq * 512:(nq + 1) * 512],
                                             psx[:, :], Exp, scale=SCALE)
                    pxgs[h] = pxg

                ps_gx = pool_ps_gx.tile([DV + 1, 2 * G], F32, tag="psgx",
                                        name="ps_gx")
                probs = {}

                def do_pv(h, i):
                    ps_o = pool_ps_o.tile([P, 130], F32, tag="ps_o", name="ps_o")
                    for dj, j in enumerate((i - 1, i, i + 1)):
                        pj, cb, jlo = probs[(h, j)]
                        c0 = cb + (i - jlo) * BLOCK
                        nc.tensor.matmul(ps_o[:, 0:65],
                                         lhsT=pj[:, c0:c0 + BLOCK],
                                         rhs=v_sb[:, _vblk(j), h, 0:65],
                                         start=(dj == 0), stop=(dj == 2))
                    nc.tensor.matmul(ps_o[:, 65:130],
                                     lhsT=pxgs[h][:, i * BLOCK:(i + 1) * BLOCK],
                                     rhs=vg_sb[:, h, 0:65],
                                     start=True, stop=True)
                    rec = pool_nrm.tile([P, 2], F32, tag="rec", name="rec")
                    dns = ps_o[:].rearrange("p (a b) -> p a b", a=2)[:, :, 64]
                    nc.vector.reciprocal(rec[:, 0:2], dns)
                    tG = pool_nrm.tile([P, DV], ATT, tag="tG", name="tG")
                    nc.vector.tensor_scalar_mul(tG[:], ps_o[:, 65:129],
                                                rec[:, 1:2])
                    nc.vector.scalar_tensor_tensor(
                        out_x[:, i, h * DV:(h + 1) * DV],
                        ps_o[:, 0:64], rec[:, 0:1], tG[:],
                        op0=mybir.AluOpType.mult, op1=mybir.AluOpType.add)

                for r_ in range(-1, 17):
                    # scores for k-block r_ for BOTH heads of the pair:
                    # adjacent MMs at partition bases 0/64 run concurrently
                    # in different PE row groups.
                    ilo, ihi = max(r_ - 1, 0), min(r_ + 1, QB - 1)
                    nloc = (ihi - ilo + 1) * BLOCK
                    own = 0 <= r_ <= 15
                    ntot = nloc + (G if own else 0)
                    kc0 = _kcols(r_)
                    ps_s = pool_ps_s.tile([P, 1024], F32, tag="ps_s", name="ps_s")
                    pt = pool_probs.tile([P, 1024], ATT, tag="probs", name="pt")
                    for sub, h in enumerate(heads):
                        cb = 512 * sub
                        nc.tensor.matmul(ps_s[:, cb:cb + nloc],
                                         lhsT=qk(kT_sb, h, kc0, kc0 + BLOCK),
                                         rhs=qk(qT_sb, h, ilo * BLOCK,
                                                (ihi + 1) * BLOCK),
                                         start=True, stop=True)
                        if own:
                            nc.tensor.matmul(ps_s[:, cb + nloc:cb + ntot],
                                             lhsT=qk(kT_sb, h, kc0, kc0 + BLOCK),
                                             rhs=qk(qT_sb, h, QG0, QG0 + G),
                                             start=True, stop=True)
                        probs[(h, r_)] = (pt, cb, ilo)
                    nc.scalar.activation(pt[:, :512 + ntot], ps_s[:, :512 + ntot],
                                         Exp, scale=SCALE)
                    if own:    # gx numerator/denominator accumulation
                        for sub, h in enumerate(heads):
                            nc.tensor.matmul(
                                ps_gx[:, sub * G:(sub + 1) * G],
                                lhsT=v_sb[:, r_, h, 0:65],
                                rhs=pt[:, 512 * sub + nloc:512 * sub + ntot],
                                start=(r_ == 0), stop=(r_ == 15))
                    i = r_ - 1
                    if 0 <= i <= QB - 1:
                        for h in heads:
                            do_pv(h, i)
                    for key in list(probs):
                        if key[1] < r_ - 2:
                            probs.pop(key)
                # stash gx partials for both heads
                for sub, h in enumerate(heads):
                    nc.vector.tensor_copy(gx_sb[:, h, :],
                                          ps_gx[:, sub * G:(sub + 1) * G])

            nc.sync.dma_start(out=gx_part_d[:], in_=gx_sb[:])
            nc.gpsimd.collective_compute(
                "AllReduce", mybir.AluOpType.add,
                replica_groups=[[0, 1, 2, 3], [4, 5, 6, 7]],
                ins=[gx_part_d.opt()], outs=[gx_full_d.opt()])

        # ---------------- phase 3: output projection ----------------
        with ExitStack() as s4:
            pool_wo = s4.enter_context(tc.tile_pool(name="wo", bufs=1))
            pool_ot = s4.enter_context(tc.tile_pool(name="ot", bufs=10))
            pool_pst = s4.enter_context(tc.tile_pool(name="pst", bufs=3, space="PSUM"))
            pool_psy = s4.enter_context(tc.tile_pool(name="psy", bufs=2, space="PSUM"))
            pool_ysb = s4.enter_context(tc.tile_pool(name="ysb", bufs=3))
            pool_gxf = s4.enter_context(tc.tile_pool(name="gxf", bufs=1))
            wo_sb = pool_wo.tile([P, KC, D_MODEL], R32)
            bo_sb = pool_wo.tile([1, D_MODEL], F32)
            ones1 = pool_wo.tile([1, P], F32)
            bias_sb = pool_wo.tile([P, D_MODEL], F32)
            ident = pool_wo.tile([P, P], ATT)
            nc.sync.dma_start(out=wo_sb[:], in_=rb(woT_d[:]))
            nc.sync.dma_start(out=bo_sb[:], in_=bo_d[:])
            nc.vector.memset(ones1[:], 1.0)
            make_identity(nc, ident[:])
            for nv in range(2):
                psb0 = pool_psy.tile([P, 512], F32, tag="psy")
                nc.tensor.matmul(psb0[:], lhsT=ones1[:],
                                 rhs=bo_sb[:, nv * 512:(nv + 1) * 512],
                                 start=True, stop=True)
                nc.scalar.activation(bias_sb[:, nv * 512:(nv + 1) * 512],
                                     psb0[:], Copy)

            for m in range(QB):
                ots = []
                for kc in range(KC):
                    pst = pool_pst.tile([P, P], ATT, tag="pst")
                    nc.tensor.transpose(pst[:],
                                        out_x[:, m, kc * P:(kc + 1) * P],
                                        ident[:])
                    ot = pool_ot.tile([P, P], R32, tag="ot")
                    nc.scalar.activation(ot[:], pst[:], Copy)
                    ots.append(ot)
                for nv in range(2):
                    psy = pool_psy.tile([P, 512], F32, tag="psy")
                    for kc in range(KC):
                        nc.tensor.matmul(psy[:],
                                         lhsT=ots[kc][:],
                                         rhs=wo_sb[:, kc, nv * 512:(nv + 1) * 512],
                                         start=(kc == 0), stop=(kc == KC - 1))
                    ysb = pool_ysb.tile([P, 512], F32, tag="ysb")
                    nc.vector.tensor_add(ysb[:], psy[:],
                                         bias_sb[:, nv * 512:(nv + 1) * 512])
                    nc.sync.dma_start(
                        out=y_own_d[m * P:(m + 1) * P, nv * 512:(nv + 1) * 512],
                        in_=ysb[:])

            # ----- global rows: normalize gx and project -----
            num_sb = pool_gxf.tile([P, KC, G], F32)     # [(h d) chunks, g]
            den_sb = pool_gxf.tile([H, G], F32)
            rden = pool_gxf.tile([H, G], F32)
            sel = pool_gxf.tile([H, H * 64], F32)
            norm_sb = pool_gxf.tile([P, KC, G], R32)
            nc.gpsimd.memset(sel[:], 0.0)
            sel3 = sel[:].rearrange("k (h d) -> k h d", h=H)
            nc.gpsimd.affine_select(
                out=sel3, in_=sel3,
                compare_op=mybir.AluOpType.not_equal, fill=1.0,
                base=0, pattern=[[-1, H], [0, 64]], channel_multiplier=1)
            for h in range(H):
                nc.sync.dma_start(
                    out=num_sb[64 * (h % 2):64 * (h % 2) + 64, h // 2, :],
                    in_=gx_full_d[0:64, h, :])
            nc.sync.dma_start(out=den_sb[:], in_=gx_full_d[64, :, :])
            nc.vector.reciprocal(rden[:], den_sb[:])
            for h in range(H):
                psb = pool_pst.tile([64, G], F32, tag="pst")
                nc.tensor.matmul(psb[:], lhsT=sel[:, h * 64:(h + 1) * 64],
                                 rhs=rden[:], start=True, stop=True)
                sl = (slice(64 * (h % 2), 64 * (h % 2) + 64), h // 2, slice(None))
                nc.vector.tensor_mul(norm_sb[sl], num_sb[sl], psb[:])
            for nv in range(2):
                psy = pool_psy.tile([G, 512], F32, tag="psy")
                for kc in range(KC):
                    nc.tensor.matmul(psy[:],
                                     lhsT=norm_sb[:, kc, :],
                                     rhs=wo_sb[:, kc, nv * 512:(nv + 1) * 512],
                                     start=(kc == 0), stop=(kc == KC - 1))
                ygsb = pool_ysb.tile([G, 512], F32, tag="ygsb")
                nc.vector.tensor_add(ygsb[:], psy[:],
                                     bias_sb[0:G, nv * 512:(nv + 1) * 512])
                nc.sync.dma_start(out=y_g_d[:, nv * 512:(nv + 1) * 512],
                                  in_=ygsb[:])

    nc.compile()
    return nc


def shard_inputs(x, Wq, Wk, Wv, Wo, bo):
    """Build the 8 per-core input maps."""
    x = np.asarray(x, dtype=np.float32)
    wqT = np.ascontiguousarray(
        np.asarray(Wq, np.float32).T.reshape(KC, P, H * DK).transpose(1, 0, 2))
    wkT = np.ascontiguousarray(
        np.asarray(Wk, np.float32).T.reshape(KC, P, H * DK).transpose(1, 0, 2))
    wvT = np.ascontiguousarray(
        np.asarray(Wv, np.float32).T.reshape(KC, P, H * DV).transpose(1, 0, 2))
    woT = np.ascontiguousarray(
        np.asarray(Wo, np.float32).T.reshape(KC, P, D_MODEL).transpose(1, 0, 2))
    bo2 = np.asarray(bo, np.float32).reshape(1, D_MODEL)
    in_maps = []
    for c in range(N_CORES):
        b, t = c // 4, c % 4
        xg = x[b, :G]                       # [16, 1024]
        xl = x[b, G:]                       # [8192, 1024]
        own = xl[t * T_OWN:(t + 1) * T_OWN]
        hl = xl[((16 * t - 1) % NBLK) * BLOCK:][:BLOCK]
        hr = xl[((16 * t + 16) % NBLK) * BLOCK:][:BLOCK]
        xc = np.concatenate([own, hl, hr, xg], axis=0)          # [2320, 1024]
        xT = np.ascontiguousarray(
            xc.T.reshape(KC, P, XC).transpose(1, 0, 2))         # [128, 8, 2320]
        in_maps.append({"xin": xT, "wqT": wqT, "wkT": wkT, "wvT": wvT,
                        "woT": woT, "bo": bo2})
    return in_maps


_NC_CACHE = {}


def get_program():
    key = (USE_F32R, ATT_BF16)
    if key not in _NC_CACHE:
        _NC_CACHE[key] = build_program()
    return _NC_CACHE[key]


def _install_ntff_hook():
    """Provide antenv.axon_hooks (missing in this image) so that
    run_bass_kernel_spmd(trace=True) can capture NTFF profiles."""
    import sys, types
    if "antenv.axon_hooks" in sys.modules:
        return
    try:
        import antenv  # noqa: F401
        from trn_agent_boot.trn_boot import _ntff_profile_via_ctypes
        mod = types.ModuleType("antenv.axon_hooks")
        mod._hook = _ntff_profile_via_ctypes("/opt/axon/libaxon_pjrt.so")
        mod.set_axon_ntff_profile_hook = lambda h: setattr(mod, "_hook", h)
        mod.get_axon_ntff_profile_hook = lambda: mod._hook
        sys.modules["antenv.axon_hooks"] = mod
    except Exception as e:  # profiling is optional
        print(f"ntff hook install failed: {e}")


def run(x, Wq, Wk, Wv, Wo, bo, trace=False):
    from concourse.bass_utils import run_bass_kernel_spmd
    if trace:
        _install_ntff_hook()
    nc = get_program()
    in_maps = shard_inputs(x, Wq, Wk, Wv, Wo, bo)
    res = run_bass_kernel_spmd(nc, in_maps, list(range(N_CORES)), trace=trace)
    y = np.empty((B, T, D_MODEL), dtype=np.float32)
    for c in range(N_CORES):
        b, t = c // 4, c % 4
        if t == 0:
            y[b, :G] = res.results[c]["y_g"]
        y[b, G + t * T_OWN:G + (t + 1) * T_OWN] = res.results[c]["y_own"]
    return y, res


def kernel(x, Wq, Wk, Wv, Wo, bo):
    y, _ = run(x, Wq, Wk, Wv, Wo, bo, trace=False)
    return y
